# revision 41
# baseline (speedup 1.0000x reference)
"""Trainium2 Bass kernel for nn_CausalSelfAttention_14980845928591.

Full-input contract: kernel(**inputs) takes the unsharded numpy inputs and
returns the full [B, T, C] output. Internally shards across 8 NeuronCores:
data-parallel over B (4 batches) x tensor-parallel over heads (2 groups of 8
heads). Causal attention is independent per (batch, head); the output
projection is a partial sum over head groups, reduced on the host.

v4 design:
  - all matmul operands bf16 (host-side casts): the PE hides bf16 weight
    loads behind streaming (fp32r self-loading matmuls cannot); PSUM
    accumulation stays f32.
  - fully SBUF-resident: x loaded once, q kept on-chip.
  - projection rounds interleaved with attention i-tiles: attention's exp
    stream (the scalar-engine bottleneck) overlaps the ACT-free QKV/V
    matmul blocks of the next t-block round.
  - attention inner loop software-pipelined: AV matmuls lag the S matmuls
    by two k-block iterations so the PE rarely waits on exp.
  - softmax row-sum reciprocals via a DRAM reshape bounce ([1,1024] ->
    [32,32]) making the DVE reciprocal ~free; partition-broadcast of the
    reciprocals also via DRAM (zero-step partition DMA needs a DRAM src).
"""
import sys

sys.path.insert(0, "/opt/trn_rl_repo")

import numpy as np
import ml_dtypes

import concourse.bass as bass
import concourse.mybir as mybir
import concourse.tile as tile
from concourse import bacc, bass_utils

# Problem shapes (hardcoded per contract).
B, T, C = 4, 2048, 1024
H, D = 16, 64
HL = 8            # heads per core
GC = HL * D       # 512: local q/k/v/gate/proj-row columns
P = 128
KC = C // P       # 8 contraction chunks
NTB = T // 512    # 4 T-blocks of 512
NIT = T // 512    # 4 i-tiles of 512
F32 = mybir.dt.float32
BF16 = mybir.dt.bfloat16

_NC_CACHE = {}


def _build_nc():
    nc = bacc.Bacc("TRN2", target_bir_lowering=False, debug=False, num_devices=8)

    xT = nc.dram_tensor("xT", [C, T], BF16, kind="ExternalInput")
    wqk = nc.dram_tensor("wqk", [C, 2 * GC], BF16, kind="ExternalInput")
    wv = nc.dram_tensor("wv", [C, GC], BF16, kind="ExternalInput")
    wg = nc.dram_tensor("wg", [C, GC], BF16, kind="ExternalInput")
    bg = nc.dram_tensor("bg", [GC], F32, kind="ExternalInput")
    wp = nc.dram_tensor("wp", [GC, C], BF16, kind="ExternalInput")
    cst = nc.dram_tensor("cst", [2, P, P], BF16, kind="ExternalInput")
    outT = nc.dram_tensor("outT", [C, T], BF16, kind="ExternalOutput")
    r_d = nc.dram_tensor("r_scratch", [NIT, 4, 1024], F32, kind="Internal")
    r_d2 = nc.dram_tensor("r_scratch2", [NIT, 4, 1024], F32, kind="Internal")

    with tile.TileContext(nc) as tc, \
         tc.tile_pool(name="pers", bufs=1) as pers, \
         tc.tile_pool(name="xsb", bufs=1) as xsbp, \
         tc.tile_pool(name="w1", bufs=1) as w1, \
         tc.tile_pool(name="pT", bufs=4) as pTp, \
         tc.tile_pool(name="og", bufs=2) as ogp, \
         tc.tile_pool(name="rr", bufs=2) as rrp, \
         tc.tile_pool(name="rbt", bufs=2) as rbtp, \
         tc.tile_pool(name="yT", bufs=2) as yTp, \
         tc.tile_pool(name="ob", bufs=2) as obp, \
         tc.tile_pool(name="ps1", bufs=2, space="PSUM") as ps1, \
         tc.tile_pool(name="ps_s", bufs=2, space="PSUM") as pss, \
         tc.tile_pool(name="ps_o", bufs=1, space="PSUM") as pso:

        # ---------- persistent SBUF tiles ----------
        kT = [pers.tile([P, T], BF16, tag=f"kT{i}", name=f"kT{i}") for i in range(4)]
        qT = [pers.tile([P, T], BF16, tag=f"qT{i}", name=f"qT{i}") for i in range(4)]
        gT = [pers.tile([P, T], F32, tag=f"gT{i}", name=f"gT{i}") for i in range(4)]
        vsb = [pers.tile([P, HL, 65], BF16, tag=f"v{j}", name=f"v{j}")
               for j in range(T // P)]
        wpsb = [pers.tile([P, C], BF16, tag=f"wp{k}", name=f"wp{k}") for k in range(4)]
        umask = pers.tile([P, P], BF16, tag="umask")
        zbias = pers.tile([P, 1], F32, tag="zbias")
        bgsb = pers.tile([P, 4], F32, tag="bg")
        x_sb = [xsbp.tile([P, T], BF16, tag=f"x{k}", name=f"x{k}")
                for k in range(KC)]
        wqksb = [w1.tile([P, 2 * GC], BF16, tag=f"wqk{k}", name=f"wqk{k}")
                 for k in range(KC)]
        wvsb = [w1.tile([P, GC], BF16, tag=f"wv{k}", name=f"wv{k}")
                for k in range(KC)]
        wgsb = [w1.tile([P, GC], BF16, tag=f"wg{k}", name=f"wg{k}")
                for k in range(KC)]

        nc.gpsimd.memset(zbias, 0.0)
        nc.sync.dma_start(out=umask, in_=cst[0])
        nc.sync.dma_start(out=bgsb, in_=bg.rearrange("(m p) -> p m", m=4))

        # prefetch order: wqk, x(tb0), wv, wg, x(tb1), wp, x(tb2), x(tb3)
        for k in range(KC):
            nc.sync.dma_start(out=wqksb[k], in_=wqk[k * P:(k + 1) * P, :])
        for k in range(KC):
            nc.sync.dma_start(out=x_sb[k][:, 0:512],
                              in_=xT[k * P:(k + 1) * P, 0:512])
        for k in range(KC):
            nc.sync.dma_start(out=wgsb[k], in_=wg[k * P:(k + 1) * P, :])
            nc.sync.dma_start(out=wvsb[k], in_=wv[k * P:(k + 1) * P, :])
        for k in range(KC):
            nc.sync.dma_start(out=x_sb[k][:, 512:1024],
                              in_=xT[k * P:(k + 1) * P, 512:1024])
        for k in range(4):
            nc.sync.dma_start(out=wpsb[k], in_=wp[k * P:(k + 1) * P, :])
        for tb in (2, 3):
            for k in range(KC):
                nc.sync.dma_start(
                    out=x_sb[k][:, tb * 512:(tb + 1) * 512],
                    in_=xT[k * P:(k + 1) * P, tb * 512:(tb + 1) * 512])

        # ---------- phase-1 building blocks ----------
        def qk_block(tb, m):
            tsl = slice(tb * 512, (tb + 1) * 512)
            ps = ps1.tile([P, 512], F32, tag="ps1", name="ps1")
            for k in range(KC):
                nc.tensor.matmul(
                    ps, wqksb[k][:, m * P:(m + 1) * P], x_sb[k][:, tsl],
                    start=(k == 0), stop=(k == KC - 1))
            if m < 4:
                nc.vector.tensor_copy(out=qT[m][:, tsl], in_=ps)
            else:
                nc.vector.tensor_copy(out=kT[m - 4][:, tsl], in_=ps)

        def gate_block(tb, m):
            tsl = slice(tb * 512, (tb + 1) * 512)
            ps = ps1.tile([P, 512], F32, tag="ps1", name="ps1")
            for k in range(KC):
                nc.tensor.matmul(
                    ps, wgsb[k][:, m * P:(m + 1) * P], x_sb[k][:, tsl],
                    start=(k == 0), stop=(k == KC - 1))
            # sigmoid(z) = 0.5*(1+tanh(z/2)): tanh shares the ACT table set
            # with exp, so the whole kernel needs zero table swaps. The
            # (1+t) shows up in the og fold; the 0.5 rides on the row-sums.
            # bgsb already holds b_gate/2 (host-side).
            nc.scalar.activation(
                gT[m][:, tsl], ps, mybir.ActivationFunctionType.Tanh,
                bias=bgsb[:, m:m + 1], scale=0.5)

        def v_block(tb, mt):
            j = tb * 4 + mt
            ps = ps1.tile([P, 512], F32, tag="ps1", name="ps1")
            for k in range(KC):
                nc.tensor.matmul(
                    ps, x_sb[k][:, j * P:(j + 1) * P], wvsb[k],
                    start=(k == 0), stop=(k == KC - 1))
            nc.vector.tensor_copy(
                out=vsb[j][:, :, 0:64],
                in_=ps.rearrange("p (h d) -> p h d", h=HL))
            nc.gpsimd.memset(vsb[j][:, :, 64:65], 1.0)

        # ---------- attention building blocks ----------
        def attn_p(it, p, r8):
            isl = slice(it * 512, (it + 1) * 512)
            njb = 4 * it + 4
            O2 = pso.tile([65, 1024], F32, tag="O2", name="O2")
            # software pipeline: S(jj) runs 2 iterations ahead of AV(jj)
            # so the PE rarely waits on the ACT-engine exp.
            pTs = {}
            for jj in range(njb + 2):
                if jj < njb:
                    jb = jj
                    jsl = slice(jb * P, (jb + 1) * P)
                    ko = jb - 4 * it
                    c0 = P * ko if ko > 0 else 0
                    s2 = pss.tile([P, 1024], F32, tag="s2", name="s2")
                    nc.tensor.matmul(s2[:, c0:512], kT[p][0:64, jsl],
                                     qT[p][0:64, it * 512 + c0:(it + 1) * 512],
                                     start=True, stop=True)
                    nc.tensor.matmul(s2[:, 512 + c0:1024],
                                     kT[p][64:128, jsl],
                                     qT[p][64:128, it * 512 + c0:(it + 1) * 512],
                                     start=True, stop=True)
                    pT = pTp.tile([P, 1024], BF16, tag="pT", name="pT")
                    if ko > 0:
                        # one ACT call over both heads' live column blocks
                        # via a strided view; skips the dead strip between
                        s2v = s2.rearrange("p (h t) -> p h t", h=2)
                        pTv = pT.rearrange("p (h t) -> p h t", h=2)
                        nc.scalar.activation(
                            pTv[:, :, c0:512], s2v[:, :, c0:512],
                            mybir.ActivationFunctionType.Exp,
                            bias=zbias, scale=0.125)
                    else:
                        nc.scalar.activation(
                            pT, s2, mybir.ActivationFunctionType.Exp,
                            bias=zbias, scale=0.125)
                    if ko >= 0:
                        # causal mask for the diagonal block
                        nc.vector.tensor_mul(pT[:, c0:c0 + P],
                                             pT[:, c0:c0 + P], umask)
                        nc.vector.tensor_mul(pT[:, 512 + c0:512 + c0 + P],
                                             pT[:, 512 + c0:512 + c0 + P],
                                             umask)
                    pTs[jb] = (pT, c0)
                if jj >= 2:
                    jb = jj - 2
                    pT, c0 = pTs.pop(jb)
                    st, sp = (jb == 0), (jb == njb - 1)
                    nc.tensor.matmul(O2[:, c0:512], vsb[jb][:, 2 * p, :],
                                     pT[:, c0:512], start=st, stop=sp,
                                     skip_group_check=True)
                    nc.tensor.matmul(O2[:, 512 + c0:1024],
                                     vsb[jb][:, 2 * p + 1, :],
                                     pT[:, 512 + c0:1024],
                                     start=st, stop=sp,
                                     skip_group_check=True)
            # O*(1+tanh) folded into the PSUM read; row-sum reciprocal; yT
            one = mybir.AluOpType.add
            mul = mybir.AluOpType.mult
            og = ogp.tile([P, 512], F32, tag="og", name="og")
            nc.vector.scalar_tensor_tensor(
                out=og[0:64, :], in0=gT[p][0:64, isl], scalar=1.0,
                in1=O2[0:64, 0:512], op0=one, op1=mul)
            nc.vector.scalar_tensor_tensor(
                out=og[64:128, :], in0=gT[p][64:128, isl], scalar=1.0,
                in1=O2[0:64, 512:1024], op0=one, op1=mul)
            # 2x on the row-sums implements the sigmoid's 0.5 factor
            nc.vector.tensor_scalar_mul(r8[32 * p:32 * p + 1, :],
                                        O2[64:65, :], 2.0)
            if it == 3 and p == 3:
                # final-tile tail: the DMA-reshape bounce is the critical
                # path here; exp(-ln(r)) on the (idle) ACT engine is faster
                r8ln = rrp.tile([P, 1024], F32, tag="r8ln", name="r8ln")
                r8rc = rrp.tile([P, 1024], F32, tag="r8rc", name="r8rc")
                zb1 = zbias[32 * p:32 * p + 1, :]
                nc.scalar.activation(
                    r8ln[32 * p:32 * p + 1, :], r8[32 * p:32 * p + 1, :],
                    mybir.ActivationFunctionType.Ln, bias=zb1)
                nc.scalar.activation(
                    r8rc[32 * p:32 * p + 1, :], r8ln[32 * p:32 * p + 1, :],
                    mybir.ActivationFunctionType.Exp, bias=zb1, scale=-1.0)
                nc.gpsimd.dma_start(out=r_d2[it, p],
                                    in_=r8rc[32 * p:32 * p + 1, :])
            else:
                # Row-sum reciprocals: a [1,1024] DVE reciprocal costs
                # ~6.5us (cost is free-size bound), so bounce through DRAM
                # reshaped to [32,32], then bounce again for the
                # partition-broadcast.
                nc.gpsimd.dma_start(out=r_d[it, p],
                                    in_=r8[32 * p:32 * p + 1, :])
                rq = rrp.tile([32, 32], F32, tag="rq", name="rq")
                rqr = rrp.tile([32, 32], F32, tag="rqr", name="rqr")
                nc.gpsimd.dma_start(
                    out=rq, in_=r_d[it, p].rearrange("(a b) -> a b", a=32))
                nc.vector.reciprocal(out=rqr, in_=rq)
                nc.gpsimd.dma_start(
                    out=r_d2[it, p].rearrange("(a b) -> a b", a=32), in_=rqr)
            rb = rbtp.tile([P, 512], F32, tag="rb", name="rb")
            for half in range(2):
                nc.gpsimd.dma_start(
                    out=rb[half * 64:(half + 1) * 64, :],
                    in_=r_d2[it, p:p + 1, half * 512:(half + 1) * 512]
                    .to_broadcast((64, 512)))
            yt = yTp.tile([P, 512], BF16, tag=f"y{p}", name=f"y{p}")
            nc.vector.tensor_mul(yt, og, rb)
            return yt

        def proj_m(it, yts, m):
            isl = slice(it * 512, (it + 1) * 512)
            ps = ps1.tile([P, 512], F32, tag="ps1", name="ps1")
            for k in range(4):
                nc.tensor.matmul(
                    ps, wpsb[k][:, m * P:(m + 1) * P], yts[k],
                    start=(k == 0), stop=(k == 3))
            ob = obp.tile([P, 512], BF16, tag="ob", name="ob")
            nc.vector.tensor_copy(out=ob, in_=ps)
            nc.sync.dma_start(out=outT[m * P:(m + 1) * P, isl], in_=ob)

        # ---------- schedule ----------
        # round 0: QK of tb0, then ALL gate blocks grouped (one ACT table
        # swap total before the exp stream starts), then V of tb0.
        for m in range(8):
            qk_block(0, m)
        for tb in range(NTB):
            for m in range(4):
                gate_block(tb, m)
        for mt in range(4):
            v_block(0, mt)
        # rounds 1..4: attention it r-1 interleaved with tb r's QK/V blocks
        # and with the projection of it r-2 (lagged one round so the
        # ACT-bound late i-tiles get ACT-free PE filler); round 5: proj it3.
        all_yts = {}
        for r in range(1, 5):
            it = r - 1
            r8 = rrp.tile([P, 1024], F32, tag="r8", name="r8")
            yts = []
            for p in range(4):
                if r <= 3:
                    qk_block(r, 2 * p)
                    qk_block(r, 2 * p + 1)
                yts.append(attn_p(it, p, r8))
                if r >= 2:
                    proj_m(it - 1, all_yts[it - 1], 2 * p)
                    proj_m(it - 1, all_yts[it - 1], 2 * p + 1)
            all_yts[it] = yts
            if r <= 3:
                for mt in range(4):
                    v_block(r, mt)
        for m in range(8):
            proj_m(3, all_yts[3], m)

    nc.compile()
    return nc


def make_in_maps(x, w_attn, w_proj, w_gate, b_gate):
    bf = ml_dtypes.bfloat16
    umask_np = np.triu(np.ones((P, P), dtype=np.float32))
    cst = np.ascontiguousarray(
        np.stack([umask_np, np.zeros((P, P), np.float32)])).astype(bf)
    x = np.asarray(x, dtype=np.float32)
    w_attn = np.asarray(w_attn, dtype=np.float32)
    w_proj = np.asarray(w_proj, dtype=np.float32)
    w_gate = np.asarray(w_gate, dtype=np.float32)
    b_gate = np.asarray(b_gate, dtype=np.float32)
    in_maps = []
    for c in range(8):
        b, g = c // 2, c % 2
        hsl = slice(g * GC, (g + 1) * GC)
        in_maps.append({
            "xT": np.ascontiguousarray(x[b].T).astype(bf),
            "wqk": np.ascontiguousarray(
                np.concatenate([w_attn[:, hsl],
                                w_attn[:, C + g * GC:C + (g + 1) * GC]],
                               axis=1)).astype(bf),
            "wv": np.ascontiguousarray(
                w_attn[:, 2 * C + g * GC:2 * C + (g + 1) * GC]).astype(bf),
            "wg": np.ascontiguousarray(w_gate[:, hsl]).astype(bf),
            # halved: the tanh-based sigmoid needs tanh((z + b)/2)
            "bg": np.ascontiguousarray(b_gate[hsl] * 0.5),
            "wp": np.ascontiguousarray(w_proj[hsl, :]).astype(bf),
            "cst": cst,
        })
    return in_maps


def kernel(x, w_attn, w_proj, w_gate, b_gate):
    if "nc" not in _NC_CACHE:
        _NC_CACHE["nc"] = _build_nc()
    nc = _NC_CACHE["nc"]

    in_maps = make_in_maps(x, w_attn, w_proj, w_gate, b_gate)
    res = bass_utils.run_bass_kernel_spmd(nc, in_maps, core_ids=list(range(8)))

    out = np.empty((B, T, C), dtype=np.float32)
    for b in range(B):
        acc = res.results[2 * b]["outT"].astype(np.float32)
        acc = acc + res.results[2 * b + 1]["outT"].astype(np.float32)
        out[b] = acc.T
    return out


# revision 45
# speedup vs baseline: 1.0630x; 1.0630x over previous
"""Trainium2 Bass kernel for nn_CausalSelfAttention_14980845928591.

Full-input contract: kernel(**inputs) takes the unsharded numpy inputs and
returns the full [B, T, C] output. Internally shards across 8 NeuronCores:
data-parallel over B (4 batches) x tensor-parallel over heads (2 groups of 8
heads). Causal attention is independent per (batch, head); the output
projection is a partial sum over head groups, reduced on the host.

v4 design:
  - all matmul operands bf16 (host-side casts): the PE hides bf16 weight
    loads behind streaming (fp32r self-loading matmuls cannot); PSUM
    accumulation stays f32.
  - fully SBUF-resident: x loaded once, q kept on-chip.
  - projection rounds interleaved with attention i-tiles: attention's exp
    stream (the scalar-engine bottleneck) overlaps the ACT-free QKV/V
    matmul blocks of the next t-block round.
  - attention inner loop software-pipelined: AV matmuls lag the S matmuls
    by two k-block iterations so the PE rarely waits on exp.
  - softmax row-sum reciprocals via a DRAM reshape bounce ([1,1024] ->
    [32,32]) making the DVE reciprocal ~free; partition-broadcast of the
    reciprocals also via DRAM (zero-step partition DMA needs a DRAM src).
"""
import sys

sys.path.insert(0, "/opt/trn_rl_repo")

import numpy as np
import ml_dtypes

import concourse.bass as bass
import concourse.mybir as mybir
import concourse.tile as tile
from concourse import bacc, bass_utils

# Problem shapes (hardcoded per contract).
B, T, C = 4, 2048, 1024
H, D = 16, 64
HL = 8            # heads per core
GC = HL * D       # 512: local q/k/v/gate/proj-row columns
P = 128
KC = C // P       # 8 contraction chunks
NTB = T // 512    # 4 T-blocks of 512
NIT = T // 512    # 4 i-tiles of 512
F32 = mybir.dt.float32
BF16 = mybir.dt.bfloat16

_NC_CACHE = {}


def _build_nc():
    nc = bacc.Bacc("TRN2", target_bir_lowering=False, debug=False, num_devices=8)

    xT = nc.dram_tensor("xT", [C, T], BF16, kind="ExternalInput")
    wqk = nc.dram_tensor("wqk", [C, 2 * GC], BF16, kind="ExternalInput")
    wv = nc.dram_tensor("wv", [C, GC], BF16, kind="ExternalInput")
    wg = nc.dram_tensor("wg", [C, GC], BF16, kind="ExternalInput")
    bg = nc.dram_tensor("bg", [GC], F32, kind="ExternalInput")
    wp = nc.dram_tensor("wp", [GC, C], BF16, kind="ExternalInput")
    cst = nc.dram_tensor("cst", [2, P, P], BF16, kind="ExternalInput")
    outT = nc.dram_tensor("outT", [C, T], BF16, kind="ExternalOutput")
    r_d = nc.dram_tensor("r_scratch", [NIT, 4, 1024], F32, kind="Internal")
    r_d2 = nc.dram_tensor("r_scratch2", [NIT, 4, 1024], F32, kind="Internal")

    with tile.TileContext(nc) as tc, \
         tc.tile_pool(name="pers", bufs=1) as pers, \
         tc.tile_pool(name="xsb", bufs=1) as xsbp, \
         tc.tile_pool(name="w1", bufs=1) as w1, \
         tc.tile_pool(name="pT", bufs=4) as pTp, \
         tc.tile_pool(name="og", bufs=2) as ogp, \
         tc.tile_pool(name="rr", bufs=2) as rrp, \
         tc.tile_pool(name="rbt", bufs=2) as rbtp, \
         tc.tile_pool(name="yT", bufs=2) as yTp, \
         tc.tile_pool(name="ob", bufs=2) as obp, \
         tc.tile_pool(name="ps1", bufs=2, space="PSUM") as ps1, \
         tc.tile_pool(name="ps_s", bufs=2, space="PSUM") as pss, \
         tc.tile_pool(name="ps_o", bufs=1, space="PSUM") as pso:

        # ---------- persistent SBUF tiles ----------
        kT = [pers.tile([P, T], BF16, tag=f"kT{i}", name=f"kT{i}") for i in range(4)]
        qT = [pers.tile([P, T], BF16, tag=f"qT{i}", name=f"qT{i}") for i in range(4)]
        gT = [pers.tile([P, T], F32, tag=f"gT{i}", name=f"gT{i}") for i in range(4)]
        vsb = [pers.tile([P, HL, 65], BF16, tag=f"v{j}", name=f"v{j}")
               for j in range(T // P)]
        wpsb = [pers.tile([P, C], BF16, tag=f"wp{k}", name=f"wp{k}") for k in range(4)]
        umask = pers.tile([P, P], BF16, tag="umask")
        zbias = pers.tile([P, 1], F32, tag="zbias")
        bgsb = pers.tile([P, 4], F32, tag="bg")
        x_sb = [xsbp.tile([P, T], BF16, tag=f"x{k}", name=f"x{k}")
                for k in range(KC)]
        wqksb = [w1.tile([P, 2 * GC], BF16, tag=f"wqk{k}", name=f"wqk{k}")
                 for k in range(KC)]
        wvsb = [w1.tile([P, GC], BF16, tag=f"wv{k}", name=f"wv{k}")
                for k in range(KC)]
        wgsb = [w1.tile([P, GC], BF16, tag=f"wg{k}", name=f"wg{k}")
                for k in range(KC)]

        nc.gpsimd.memset(zbias, 0.0)
        nc.sync.dma_start(out=umask, in_=cst[0])
        nc.sync.dma_start(out=bgsb, in_=bg.rearrange("(m p) -> p m", m=4))

        # prefetch order: wqk, x(tb0), wv, wg, x(tb1), wp, x(tb2), x(tb3)
        for k in range(KC):
            nc.sync.dma_start(out=wqksb[k], in_=wqk[k * P:(k + 1) * P, :])
        for k in range(KC):
            nc.sync.dma_start(out=x_sb[k][:, 0:512],
                              in_=xT[k * P:(k + 1) * P, 0:512])
        for k in range(KC):
            nc.sync.dma_start(out=wvsb[k], in_=wv[k * P:(k + 1) * P, :])
            nc.sync.dma_start(out=wgsb[k], in_=wg[k * P:(k + 1) * P, :])
        for k in range(KC):
            nc.sync.dma_start(out=x_sb[k][:, 512:1024],
                              in_=xT[k * P:(k + 1) * P, 512:1024])
        for k in range(4):
            nc.sync.dma_start(out=wpsb[k], in_=wp[k * P:(k + 1) * P, :])
        for tb in (2, 3):
            for k in range(KC):
                nc.sync.dma_start(
                    out=x_sb[k][:, tb * 512:(tb + 1) * 512],
                    in_=xT[k * P:(k + 1) * P, tb * 512:(tb + 1) * 512])

        # ---------- phase-1 building blocks ----------
        def qk_block(tb, m):
            tsl = slice(tb * 512, (tb + 1) * 512)
            ps = ps1.tile([P, 512], F32, tag="ps1", name="ps1")
            for k in range(KC):
                nc.tensor.matmul(
                    ps, wqksb[k][:, m * P:(m + 1) * P], x_sb[k][:, tsl],
                    start=(k == 0), stop=(k == KC - 1))
            if m < 4:
                nc.vector.tensor_copy(out=qT[m][:, tsl], in_=ps)
            else:
                nc.vector.tensor_copy(out=kT[m - 4][:, tsl], in_=ps)

        def gate_block(tb, m):
            tsl = slice(tb * 512, (tb + 1) * 512)
            ps = ps1.tile([P, 512], F32, tag="ps1", name="ps1")
            for k in range(KC):
                nc.tensor.matmul(
                    ps, wgsb[k][:, m * P:(m + 1) * P], x_sb[k][:, tsl],
                    start=(k == 0), stop=(k == KC - 1))
            # sigmoid(z) = 0.5*(1+tanh(z/2)): tanh shares the ACT table set
            # with exp, so the whole kernel needs zero table swaps. The
            # (1+t) shows up in the og fold; the 0.5 rides on the row-sums.
            # bgsb already holds b_gate/2 (host-side).
            nc.scalar.activation(
                gT[m][:, tsl], ps, mybir.ActivationFunctionType.Tanh,
                bias=bgsb[:, m:m + 1], scale=0.5)

        def v_block(tb, mt):
            j = tb * 4 + mt
            ps = ps1.tile([P, 512], F32, tag="ps1", name="ps1")
            for k in range(KC):
                nc.tensor.matmul(
                    ps, x_sb[k][:, j * P:(j + 1) * P], wvsb[k],
                    start=(k == 0), stop=(k == KC - 1))
            nc.vector.tensor_copy(
                out=vsb[j][:, :, 0:64],
                in_=ps.rearrange("p (h d) -> p h d", h=HL))
            nc.gpsimd.memset(vsb[j][:, :, 64:65], 1.0)

        # ---------- attention building blocks ----------
        def attn_p(it, p, r8):
            isl = slice(it * 512, (it + 1) * 512)
            njb = 4 * it + 4
            O2 = pso.tile([65, 1024], F32, tag="O2", name="O2")
            # software pipeline: S(jj) runs 2 iterations ahead of AV(jj)
            # so the PE rarely waits on the ACT-engine exp.
            pTs = {}
            for jj in range(njb + 2):
                if jj < njb:
                    jb = jj
                    jsl = slice(jb * P, (jb + 1) * P)
                    ko = jb - 4 * it
                    c0 = P * ko if ko > 0 else 0
                    s2 = pss.tile([P, 1024], F32, tag="s2", name="s2")
                    nc.tensor.matmul(s2[:, c0:512], kT[p][0:64, jsl],
                                     qT[p][0:64, it * 512 + c0:(it + 1) * 512],
                                     start=True, stop=True)
                    nc.tensor.matmul(s2[:, 512 + c0:1024],
                                     kT[p][64:128, jsl],
                                     qT[p][64:128, it * 512 + c0:(it + 1) * 512],
                                     start=True, stop=True)
                    pT = pTp.tile([P, 1024], BF16, tag="pT", name="pT")
                    if ko > 0:
                        # one ACT call over both heads' live column blocks
                        # via a strided view; skips the dead strip between
                        s2v = s2.rearrange("p (h t) -> p h t", h=2)
                        pTv = pT.rearrange("p (h t) -> p h t", h=2)
                        nc.scalar.activation(
                            pTv[:, :, c0:512], s2v[:, :, c0:512],
                            mybir.ActivationFunctionType.Exp,
                            bias=zbias, scale=0.125)
                    else:
                        nc.scalar.activation(
                            pT, s2, mybir.ActivationFunctionType.Exp,
                            bias=zbias, scale=0.125)
                    if ko >= 0:
                        # causal mask for the diagonal block
                        nc.vector.tensor_mul(pT[:, c0:c0 + P],
                                             pT[:, c0:c0 + P], umask)
                        nc.vector.tensor_mul(pT[:, 512 + c0:512 + c0 + P],
                                             pT[:, 512 + c0:512 + c0 + P],
                                             umask)
                    pTs[jb] = (pT, c0)
                if jj >= 2:
                    jb = jj - 2
                    pT, c0 = pTs.pop(jb)
                    st, sp = (jb == 0), (jb == njb - 1)
                    nc.tensor.matmul(O2[:, c0:512], vsb[jb][:, 2 * p, :],
                                     pT[:, c0:512], start=st, stop=sp,
                                     skip_group_check=True)
                    nc.tensor.matmul(O2[:, 512 + c0:1024],
                                     vsb[jb][:, 2 * p + 1, :],
                                     pT[:, 512 + c0:1024],
                                     start=st, stop=sp,
                                     skip_group_check=True)
            # O*(1+tanh) folded into the PSUM read; row-sum reciprocal; yT
            one = mybir.AluOpType.add
            mul = mybir.AluOpType.mult
            og = ogp.tile([P, 512], F32, tag="og", name="og")
            nc.vector.scalar_tensor_tensor(
                out=og[0:64, :], in0=gT[p][0:64, isl], scalar=1.0,
                in1=O2[0:64, 0:512], op0=one, op1=mul)
            nc.vector.scalar_tensor_tensor(
                out=og[64:128, :], in0=gT[p][64:128, isl], scalar=1.0,
                in1=O2[0:64, 512:1024], op0=one, op1=mul)
            nc.vector.tensor_copy(out=r8[32 * p:32 * p + 1, :],
                                  in_=O2[64:65, :])
            if it == 3 and p == 3:
                # final-tile tail: the DMA-reshape bounce is the critical
                # path here; exp(-ln(r)) on the (idle) ACT engine is faster
                r8ln = rrp.tile([P, 1024], F32, tag="r8ln", name="r8ln")
                r8rc = rrp.tile([P, 1024], F32, tag="r8rc", name="r8rc")
                zb1 = zbias[32 * p:32 * p + 1, :]
                nc.scalar.activation(
                    r8ln[32 * p:32 * p + 1, :], r8[32 * p:32 * p + 1, :],
                    mybir.ActivationFunctionType.Ln, bias=zb1)
                nc.scalar.activation(
                    r8rc[32 * p:32 * p + 1, :], r8ln[32 * p:32 * p + 1, :],
                    mybir.ActivationFunctionType.Exp, bias=zb1, scale=-1.0)
                nc.gpsimd.dma_start(out=r_d2[it, p],
                                    in_=r8rc[32 * p:32 * p + 1, :])
            else:
                # Row-sum reciprocals: a [1,1024] DVE reciprocal costs
                # ~6.5us (cost is free-size bound), so bounce through DRAM
                # reshaped to [32,32], then bounce again for the
                # partition-broadcast.
                nc.gpsimd.dma_start(out=r_d[it, p],
                                    in_=r8[32 * p:32 * p + 1, :])
                rq = rrp.tile([32, 32], F32, tag="rq", name="rq")
                rqr = rrp.tile([32, 32], F32, tag="rqr", name="rqr")
                nc.gpsimd.dma_start(
                    out=rq, in_=r_d[it, p].rearrange("(a b) -> a b", a=32))
                nc.vector.reciprocal(out=rqr, in_=rq)
                nc.gpsimd.dma_start(
                    out=r_d2[it, p].rearrange("(a b) -> a b", a=32), in_=rqr)
            rb = rbtp.tile([P, 512], F32, tag="rb", name="rb")
            for half in range(2):
                nc.gpsimd.dma_start(
                    out=rb[half * 64:(half + 1) * 64, :],
                    in_=r_d2[it, p:p + 1, half * 512:(half + 1) * 512]
                    .to_broadcast((64, 512)))
            yt = yTp.tile([P, 512], BF16, tag=f"y{p}", name=f"y{p}")
            # the 0.5 completes sigmoid = 0.5*(1+tanh(z/2))
            nc.vector.scalar_tensor_tensor(
                out=yt, in0=og, scalar=0.5, in1=rb, op0=mul, op1=mul)
            return yt

        def proj_m(it, yts, m):
            isl = slice(it * 512, (it + 1) * 512)
            ps = ps1.tile([P, 512], F32, tag="ps1", name="ps1")
            for k in range(4):
                nc.tensor.matmul(
                    ps, wpsb[k][:, m * P:(m + 1) * P], yts[k],
                    start=(k == 0), stop=(k == 3))
            ob = obp.tile([P, 512], BF16, tag="ob", name="ob")
            nc.vector.tensor_copy(out=ob, in_=ps)
            nc.sync.dma_start(out=outT[m * P:(m + 1) * P, isl], in_=ob)

        # ---------- schedule ----------
        # round 0: QK + V of tb0 only (small: the exp stream starts early).
        # round r (1..4): gate(tb r-1) [tanh shares exp's ACT table set],
        # QK/V of tb r, attention it r-1, and the projection of it r-2
        # (lagged one round so the ACT-bound late i-tiles get ACT-free PE
        # filler); round 5: proj it3.
        for m in range(8):
            qk_block(0, m)
        for mt in range(4):
            v_block(0, mt)
        all_yts = {}
        for r in range(1, 5):
            it = r - 1
            r8 = rrp.tile([P, 1024], F32, tag="r8", name="r8")
            yts = []
            for p in range(4):
                gate_block(it, p)
                if r <= 3:
                    qk_block(r, 2 * p)
                    qk_block(r, 2 * p + 1)
                yts.append(attn_p(it, p, r8))
                if r >= 2:
                    proj_m(it - 1, all_yts[it - 1], 2 * p)
                    proj_m(it - 1, all_yts[it - 1], 2 * p + 1)
            all_yts[it] = yts
            if r <= 3:
                for mt in range(4):
                    v_block(r, mt)
        for m in range(8):
            proj_m(3, all_yts[3], m)

    nc.compile()
    return nc


def make_in_maps(x, w_attn, w_proj, w_gate, b_gate):
    bf = ml_dtypes.bfloat16
    umask_np = np.triu(np.ones((P, P), dtype=np.float32))
    cst = np.ascontiguousarray(
        np.stack([umask_np, np.zeros((P, P), np.float32)])).astype(bf)
    x = np.asarray(x, dtype=np.float32)
    w_attn = np.asarray(w_attn, dtype=np.float32)
    w_proj = np.asarray(w_proj, dtype=np.float32)
    w_gate = np.asarray(w_gate, dtype=np.float32)
    b_gate = np.asarray(b_gate, dtype=np.float32)
    in_maps = []
    for c in range(8):
        b, g = c // 2, c % 2
        hsl = slice(g * GC, (g + 1) * GC)
        in_maps.append({
            "xT": np.ascontiguousarray(x[b].T).astype(bf),
            "wqk": np.ascontiguousarray(
                np.concatenate([w_attn[:, hsl],
                                w_attn[:, C + g * GC:C + (g + 1) * GC]],
                               axis=1)).astype(bf),
            "wv": np.ascontiguousarray(
                w_attn[:, 2 * C + g * GC:2 * C + (g + 1) * GC]).astype(bf),
            "wg": np.ascontiguousarray(w_gate[:, hsl]).astype(bf),
            # halved: the tanh-based sigmoid needs tanh((z + b)/2)
            "bg": np.ascontiguousarray(b_gate[hsl] * 0.5),
            "wp": np.ascontiguousarray(w_proj[hsl, :]).astype(bf),
            "cst": cst,
        })
    return in_maps


def kernel(x, w_attn, w_proj, w_gate, b_gate):
    if "nc" not in _NC_CACHE:
        _NC_CACHE["nc"] = _build_nc()
    nc = _NC_CACHE["nc"]

    in_maps = make_in_maps(x, w_attn, w_proj, w_gate, b_gate)
    res = bass_utils.run_bass_kernel_spmd(nc, in_maps, core_ids=list(range(8)))

    out = np.empty((B, T, C), dtype=np.float32)
    for b in range(B):
        acc = res.results[2 * b]["outT"].astype(np.float32)
        acc = acc + res.results[2 * b + 1]["outT"].astype(np.float32)
        out[b] = acc.T
    return out
